# revision 15
# baseline (speedup 1.0000x reference)
"""Detection-loss Bass kernel builder (shared by dev test and final kernel.py).

Layout: per image, anchors n = p*512 + f  (p in [0,128), f in [0,512)).
Each core processes IMGS=2 images; output = sum over its images of
(cls_loss + reg_loss).  Host divides the 8 partial sums by B=16.

Container-specific constraints honored here:
 - walrus supports only ONE sync-wait per instruction -> kernel.py splits
   multi-wait instructions into NoOp chains at BIR-JSON level.
 - extended q7 ops (partition_broadcast/all_reduce, local_scatter) are NOT
   supported -> use DMA broadcast (step-0 partition APs), PE-matmul column
   sums, and indirect-DMA gathers instead.
"""
import numpy as np
import concourse.bass as bass
import concourse.mybir as mybir
import concourse.bass_isa as bass_isa

F32 = mybir.dt.float32
I16 = mybir.dt.int16
I32 = mybir.dt.int32
U32 = mybir.dt.uint32
Alu = mybir.AluOpType
Act = mybir.ActivationFunctionType

P = 128
F = 512
N = P * F          # 65536
C = 21
M = 20
CC = 16            # pos candidates per partition
NW = 8             # neg candidates per partition
IMGS = 2           # images per core
PK = 6             # packed payload slots per anchor

POS_T, NEG_T = 0.5 / 1.5, 0.4 / 1.4   # r-space: r = inter/(An+Am), iou = r/(1-r)
ALPHA = 0.25
MIN_POS = 10.0
RATIO = 3.0
MR_IMM = -1.0e30   # match_replace fill


def build(nc, tc, outs, ins):
    v = nc.vector
    g = nc.gpsimd
    s = nc.scalar
    (o_part,) = outs
    d_cls, d_reg, d_anch, d_tb, d_tl, d_sc = ins

    with tc.tile_pool(name="main", bufs=1) as pl, \
         tc.tile_pool(name="io", bufs=1) as pio, \
         tc.tile_pool(name="lp", bufs=2) as plp, \
         tc.tile_pool(name="ps", bufs=2, space="PSUM") as pps, \
         tc.tile_pool(name="dr", bufs=1, space="DRAM") as pdr:

        # ---------- one-time setup ----------
        anch = pl.tile([P, 2048], F32, tag="anch")
        nc.sync.dma_start(anch[:], d_anch.rearrange("(p f) -> p f", p=P))
        a0 = anch[:, 0:2048:4]
        a1 = anch[:, 1:2048:4]
        a2 = anch[:, 2:2048:4]
        a3 = anch[:, 3:2048:4]
        aw = pl.tile([P, F], F32, tag="aw")
        ah = pl.tile([P, F], F32, tag="ah")
        ax = pl.tile([P, F], F32, tag="ax")
        ay = pl.tile([P, F], F32, tag="ay")
        v.tensor_tensor(aw[:], a2, a0, op=Alu.subtract)
        v.tensor_tensor(ah[:], a3, a1, op=Alu.subtract)
        v.scalar_tensor_tensor(ax[:], aw[:], 0.5, a0, op0=Alu.mult, op1=Alu.add)
        v.scalar_tensor_tensor(ay[:], ah[:], 0.5, a1, op0=Alu.mult, op1=Alu.add)

        ones = pl.tile([P, 1], F32, tag="ones")
        v.memset(ones[:], 1.0)
        iota_m_i = pl.tile([P, M], I32, tag="iomi")
        g.iota(iota_m_i[:], pattern=[[1, M]], base=0, channel_multiplier=0)
        iota_m = pl.tile([P, M], F32, tag="iom")        # 0..19 f32
        v.tensor_copy(iota_m[:], iota_m_i[:])
        iota_r_i = pl.tile([P, P], I32, tag="iori")
        g.iota(iota_r_i[:], pattern=[[1, P]], base=0, channel_multiplier=0)
        iota_r = pl.tile([P, P], F32, tag="ior")        # 0..127 f32
        v.tensor_copy(iota_r[:], iota_r_i[:])
        iota_f_i = pl.tile([P, F], I32, tag="iofi")
        g.iota(iota_f_i[:], pattern=[[1, F]], base=0, channel_multiplier=0)
        iota_f = pl.tile([P, F], F32, tag="iof")        # 0..511 f32
        v.tensor_copy(iota_f[:], iota_f_i[:])

        acc_part = pl.tile([P, 1], F32, tag="accp")     # per-core result accum
        v.memset(acc_part[:], 0.0)

        def psum_bcast(dst, src_cols, n):
            """dst[128, n] = column sums of src_cols[128, n], replicated."""
            pst = pps.tile([1, 8], F32, tag="pst")
            nc.tensor.matmul(pst[:, 0:n], ones[:], src_cols)
            row = pio.tile([1, 8], F32, tag="psrow")
            v.tensor_copy(row[:, 0:n], pst[:, 0:n])
            drow = pdr.tile([1, 8], F32, tag="psdr")
            nc.sync.dma_start(drow[:, 0:n], row[:, 0:n])
            nc.sync.dma_start(dst, drow[:, 0:n].broadcast_to([P, n]))

        for img in range(IMGS):
            # ---------- loads ----------
            regs = pio.tile([P, 4, F], F32, tag="regs")
            nc.sync.dma_start(
                regs[:], d_reg[img, :, :].rearrange("r (p f) -> p r f", p=P))
            cls = pio.tile([P, C, F], F32, tag="cls")
            nc.sync.dma_start(
                cls[:], d_cls[img, :, :].rearrange("c (p f) -> p c f", p=P))
            sc = pio.tile([P, F], F32, tag="sc")
            nc.sync.dma_start(sc[:], d_sc[img, :].rearrange("(p f) -> p f", p=P))
            bgt = pio.tile([P, 80], F32, tag="bgt")     # gt boxes replicated
            nc.sync.dma_start(
                bgt[:],
                d_tb[img, :, :].rearrange("m c -> (m c)")[None, :].broadcast_to([P, 80]))
            tli = pio.tile([1, M], I32, tag="tli")
            nc.sync.dma_start(tli[:], d_tl[img, :][None, :])
            tlf0 = pio.tile([1, M], F32, tag="tlf0")
            v.tensor_copy(tlf0[:], tli[:])
            dtl = pdr.tile([1, M], F32, tag="dtl")
            nc.sync.dma_start(dtl[:], tlf0[:])
            tlf = pio.tile([P, M], F32, tag="tlf")      # labels replicated f32
            nc.sync.dma_start(tlf[:], dtl[:].broadcast_to([P, M]))

            bx0 = bgt[:, 0:80:4]
            by0 = bgt[:, 1:80:4]
            bx1 = bgt[:, 2:80:4]
            by1 = bgt[:, 3:80:4]
            bw = pio.tile([P, M], F32, tag="bw")
            bh = pio.tile([P, M], F32, tag="bh")
            bA = pio.tile([P, M], F32, tag="bA")
            v.tensor_tensor(bw[:], bx1, bx0, op=Alu.subtract)
            v.tensor_tensor(bh[:], by1, by0, op=Alu.subtract)
            v.tensor_tensor(bA[:], bw[:], bh[:], op=Alu.mult)

            # ---------- decode ----------
            dx0t = pio.tile([P, F], F32, tag="dx0t")
            dx1t = pio.tile([P, F], F32, tag="dx1t")
            dy0t = pio.tile([P, F], F32, tag="dy0t")
            dy1t = pio.tile([P, F], F32, tag="dy1t")
            Ant = pio.tile([P, F], F32, tag="Ant")
            lset = pio.tile([P, F], F32, tag="lset")
            dx0 = dx0t[:]
            dx1 = dx1t[:]
            dy0 = dy0t[:]
            dy1 = dy1t[:]
            An = Ant[:]
            lse = lset[:]
            ew = pio.tile([P, F], F32, tag="ew")
            eh = pio.tile([P, F], F32, tag="eh")
            s.activation(ew[:], regs[:, 2, :], Act.Exp)
            s.activation(eh[:], regs[:, 3, :], Act.Exp)
            w = pio.tile([P, F], F32, tag="w")
            h = pio.tile([P, F], F32, tag="h")
            v.tensor_tensor(w[:], aw[:], ew[:], op=Alu.mult)
            v.tensor_tensor(h[:], ah[:], eh[:], op=Alu.mult)
            cx = pio.tile([P, F], F32, tag="cx")
            cy = pio.tile([P, F], F32, tag="cy")
            v.tensor_tensor(cx[:], regs[:, 0, :], aw[:], op=Alu.mult)
            v.tensor_tensor(cy[:], regs[:, 1, :], ah[:], op=Alu.mult)
            v.tensor_tensor(cx[:], cx[:], ax[:], op=Alu.add)
            v.tensor_tensor(cy[:], cy[:], ay[:], op=Alu.add)
            v.scalar_tensor_tensor(dx0, w[:], -0.5, cx[:], op0=Alu.mult, op1=Alu.add)
            v.scalar_tensor_tensor(dx1, w[:], 0.5, cx[:], op0=Alu.mult, op1=Alu.add)
            v.scalar_tensor_tensor(dy0, h[:], -0.5, cy[:], op0=Alu.mult, op1=Alu.add)
            v.scalar_tensor_tensor(dy1, h[:], 0.5, cy[:], op0=Alu.mult, op1=Alu.add)
            v.tensor_tensor(An, w[:], h[:], op=Alu.mult)

            # ---------- dense IoU max over M ----------
            mx = pio.tile([P, F], F32, tag="mx")
            v.memset(mx[:], 0.0)
            un = pio.tile([P, F], F32, tag="un")
            ur = pio.tile([P, F], F32, tag="ur")
            iwc = pio.tile([P, F], F32, tag="iwc")
            ihc = pio.tile([P, F], F32, tag="ihc")
            for m in range(M):
                ix0 = plp.tile([P, F], F32, tag="ix0")
                ix1 = plp.tile([P, F], F32, tag="ix1")
                iy0 = plp.tile([P, F], F32, tag="iy0")
                iy1 = plp.tile([P, F], F32, tag="iy1")
                rm = plp.tile([P, F], F32, tag="rm")
                iw = plp.tile([P, F], F32, tag="iw")
                ih = plp.tile([P, F], F32, tag="ih")
                inter = plp.tile([P, F], F32, tag="inter")
                v.tensor_scalar(ix0[:], dx0, bx0[:, m:m + 1], None, op0=Alu.max)
                v.tensor_scalar(ix1[:], dx1, bx1[:, m:m + 1], None, op0=Alu.min)
                g.tensor_tensor(iw[:], ix1[:], ix0[:], op=Alu.subtract)
                v.tensor_scalar(iy0[:], dy0, by0[:, m:m + 1], None, op0=Alu.max)
                v.tensor_scalar(iy1[:], dy1, by1[:, m:m + 1], None, op0=Alu.min)
                g.tensor_tensor(ih[:], iy1[:], iy0[:], op=Alu.subtract)
                s.activation(iwc[:], iw[:], Act.Relu)
                s.activation(ihc[:], ih[:], Act.Relu)
                v.tensor_tensor(inter[:], iwc[:], ihc[:], op=Alu.mult)
                v.tensor_scalar(un[:], An, bA[:, m:m + 1], None, op0=Alu.add)
                v.reciprocal(ur[:], un[:])
                g.tensor_tensor(rm[:], inter[:], ur[:], op=Alu.mult)
                v.tensor_tensor(mx[:], mx[:], rm[:], op=Alu.max)

            # ---------- LSE (no max-subtraction needed; |cls| < 6) ----------
            esum = pio.tile([P, F], F32, tag="esum")
            nchnk = 7
            for ci in range(nchnk):
                c0 = 3 * ci
                echnk = plp.tile([P, 3, F], F32, tag="echnk")
                s.activation(echnk[:], cls[:, c0:c0 + 3, :], Act.Exp)
                esrc = echnk[:].rearrange("p c f -> p f c")
                if ci == 0:
                    v.tensor_reduce(esum[:], esrc, axis=mybir.AxisListType.X, op=Alu.add)
                else:
                    epart = pio.tile([P, F], F32, tag="epart")
                    v.tensor_reduce(epart[:], esrc, axis=mybir.AxisListType.X, op=Alu.add)
                    g.tensor_tensor(esum[:], esum[:], epart[:], op=Alu.add)
            s.activation(lse, esum[:], Act.Ln)

            # ---------- counts ----------
            posr = pio.tile([P, F], F32, tag="junkF")
            negm = pio.tile([P, F], F32, tag="negm")
            cnt2 = pio.tile([P, 2], F32, tag="cnt2")
            v.tensor_scalar(posr[:], mx[:], POS_T, None, op0=Alu.is_ge,
                            op1=Alu.add, accum_out=cnt2[:, 0:1])
            v.tensor_scalar(negm[:], mx[:], NEG_T, None, op0=Alu.is_lt,
                            op1=Alu.add, accum_out=cnt2[:, 1:2])
            cnt2r = pio.tile([P, 2], F32, tag="cnt2r")
            psum_bcast(cnt2r[:], cnt2[:], 2)
            npos_raw = cnt2r[:, 0:1]
            nneg = cnt2r[:, 1:2]
            use_fb = pio.tile([P, 1], F32, tag="usefb")
            v.tensor_scalar(use_fb[:], npos_raw, MIN_POS, None, op0=Alu.is_lt)
            num_pos = pio.tile([P, 1], F32, tag="numpos")
            t1 = pio.tile([P, 1], F32, tag="t1")
            v.tensor_scalar(t1[:], npos_raw, -1.0, MIN_POS, op0=Alu.mult, op1=Alu.add)
            v.tensor_tensor(t1[:], t1[:], use_fb[:], op=Alu.mult)
            v.tensor_tensor(num_pos[:], npos_raw, t1[:], op=Alu.add)
            kk = pio.tile([P, 1], F32, tag="kk")
            v.tensor_scalar(kk[:], num_pos[:], RATIO, None, op0=Alu.mult)

            # ---------- pos candidates: top-16 mx per partition ----------
            mxc = pio.tile([P, F], F32, tag="mxc")
            v.tensor_copy(mxc[:], mx[:])
            V = pio.tile([P, CC], F32, tag="V")
            I = pio.tile([P, CC], U32, tag="I")
            v.max(V[:, 0:8], mxc[:])
            v.max_index(I[:, 0:8], V[:, 0:8], mxc[:])
            mxc2 = pio.tile([P, F], F32, tag="mxc2")
            v.match_replace(mxc2[:], V[:, 0:8], mxc[:], MR_IMM)
            v.max(V[:, 8:16], mxc2[:])
            v.max_index(I[:, 8:16], V[:, 8:16], mxc2[:])

            # ---------- global candidate ranks (vs top-8 pool) + v10 ----------
            vdr = pdr.tile([P, 8], F32, tag="vdr")
            nc.sync.dma_start(vdr[:], V[:, 0:8])
            vpool = pio.tile([P, P * 8], F32, tag="pool8")
            nc.sync.dma_start(
                vpool[:],
                vdr[:].rearrange("p j -> (p j)")[None, :].broadcast_to([P, P * 8]))
            rnk = pio.tile([P, CC], F32, tag="rnk")
            for j in range(12):
                scr = plp.tile([P, P * 8], F32, tag="scrj")
                v.tensor_scalar(scr[:], vpool[:], V[:, j:j + 1], None,
                                op0=Alu.is_gt, op1=Alu.add, accum_out=rnk[:, j:j + 1])
            oh10 = pio.tile([P, CC], F32, tag="oh10")
            v.tensor_scalar(oh10[:, 0:12], rnk[:, 0:12], 9.0, None, op0=Alu.is_equal)
            pv2 = pio.tile([P, 2], F32, tag="pv2")
            scrd = pio.tile([P, 16], F32, tag="scrd")
            v.scalar_tensor_tensor(scrd[:, 0:12], oh10[:, 0:12], 1.0, V[:, 0:12],
                                   op0=Alu.mult, op1=Alu.mult, accum_out=pv2[:, 0:1])

            # ---------- neg selection threshold ----------
            vneg = pio.tile([P, F], F32, tag="vneg")
            t2 = pio.tile([P, F], F32, tag="t2")
            v.tensor_scalar(t2[:], negm[:], 2.0, -2.0, op0=Alu.mult, op1=Alu.add)
            v.tensor_tensor(vneg[:], t2[:], sc[:], op=Alu.subtract)
            W = pio.tile([P, NW], F32, tag="W")
            v.max(W[:], vneg[:])
            wdr = pdr.tile([P, NW], F32, tag="wdr")
            nc.sync.dma_start(wdr[:], W[:])
            wpool = pio.tile([P, P * NW], F32, tag="pool8")
            nc.sync.dma_start(
                wpool[:],
                wdr[:].rearrange("p j -> (p j)")[None, :].broadcast_to([P, P * NW]))
            wr = pio.tile([P, NW], F32, tag="wr")
            for j in range(NW):
                wscr = plp.tile([P, P * 8], F32, tag="scrj")
                v.tensor_scalar(wscr[:, 0:P * NW], wpool[:], W[:, j:j + 1], None,
                                op0=Alu.is_gt, op1=Alu.add, accum_out=wr[:, j:j + 1])
            km1 = pio.tile([P, 1], F32, tag="km1")
            v.tensor_scalar(km1[:], kk[:], -1.0, None, op0=Alu.add)
            ohw = pio.tile([P, NW], F32, tag="ohw")
            v.tensor_scalar(ohw[:], wr[:], km1[:], None, op0=Alu.is_equal)
            v.scalar_tensor_tensor(scrd[:, 0:NW], ohw[:], 1.0, W[:],
                                   op0=Alu.mult, op1=Alu.mult, accum_out=pv2[:, 1:2])
            pv2r = pio.tile([P, 2], F32, tag="pv2r")
            psum_bcast(pv2r[:], pv2[:], 2)
            v10 = pv2r[:, 0:1]
            tauv = pv2r[:, 1:2]
            taup = pio.tile([P, 1], F32, tag="taup")
            v.tensor_scalar(t1[:], v10, -POS_T, None, op0=Alu.add)
            v.tensor_tensor(t1[:], t1[:], use_fb[:], op=Alu.mult)
            v.tensor_scalar(taup[:], t1[:], POS_T, None, op0=Alu.add)

            # ---------- dense neg focal ----------
            ce_n = pio.tile([P, F], F32, tag="cen")
            v.tensor_tensor(ce_n[:], lse, cls[:, 0, :], op=Alu.subtract)
            pt_n = pio.tile([P, F], F32, tag="ptn")
            s.activation(pt_n[:], ce_n[:], Act.Exp, scale=-1.0)
            u_n = pio.tile([P, F], F32, tag="un2")
            v.tensor_scalar(u_n[:], pt_n[:], -1.0, 1.0, op0=Alu.mult, op1=Alu.add)
            u2_n = pio.tile([P, F], F32, tag="u2n")
            s.activation(u2_n[:], u_n[:], Act.Square)
            foc_n = pio.tile([P, F], F32, tag="focn")
            v.scalar_tensor_tensor(foc_n[:], u2_n[:], ALPHA, ce_n[:],
                                   op0=Alu.mult, op1=Alu.mult)
            sums = pio.tile([P, 4], F32, tag="sums")
            selm = pio.tile([P, F], F32, tag="selm")
            v.tensor_scalar(selm[:], vneg[:], tauv, None, op0=Alu.is_ge)
            v.scalar_tensor_tensor(selm[:], selm[:], 1.0, foc_n[:],
                                   op0=Alu.mult, op1=Alu.mult, accum_out=sums[:, 0:1])
            allm = pio.tile([P, F], F32, tag="allm")
            v.scalar_tensor_tensor(allm[:], negm[:], 1.0, foc_n[:],
                                   op0=Alu.mult, op1=Alu.mult, accum_out=sums[:, 1:2])

            # ---------- route top-128 candidates to partition slots via PE ----------
            If = pio.tile([P, CC], F32, tag="If")
            v.tensor_copy(If[:], I[:])
            OH1 = pio.tile([P, P], F32, tag="OH1")
            Wf = pio.tile([P, P], F32, tag="Wf")
            v.memset(OH1[:], 0.0)
            v.memset(Wf[:], 0.0)
            for j in range(12):
                ohj = plp.tile([P, P], F32, tag="ohj")
                v.tensor_scalar(ohj[:], iota_r[:], rnk[:, j:j + 1], None,
                                op0=Alu.is_equal)
                v.tensor_tensor(OH1[:], OH1[:], ohj[:], op=Alu.add)
                v.scalar_tensor_tensor(Wf[:], ohj[:], If[:, j:j + 1], Wf[:],
                                       op0=Alu.mult, op1=Alu.add)
            # fsel[r] = f-index of rank-r candidate (column sums of Wf)
            psF = pps.tile([1, P], F32, tag="psF")
            nc.tensor.matmul(psF[:], ones[:], Wf[:])
            rowF = pio.tile([1, P], F32, tag="rowF")
            v.tensor_copy(rowF[:], psF[:])
            dF = pdr.tile([1, P], F32, tag="dF")
            nc.sync.dma_start(dF[:], rowF[:])
            fsel = pio.tile([P, 1], F32, tag="fsel")
            nc.sync.dma_start(
                fsel[:], dF[:].rearrange("o p -> (o p)").rearrange("(p o) -> p o", o=1))
            # stage A: permute payload rows (candidate r -> partition r)
            srcs = (mx[:], dx0, dx1, dy0, dy1, An, lse)
            ohf = pio.tile([P, F], F32, tag="ohf")
            v.tensor_scalar(ohf[:], iota_f[:], fsel[:], None, op0=Alu.is_equal)
            cX = pio.tile([P, 8], F32, tag="cX")
            junk2 = pio.tile([P, F], F32, tag="junkF")
            for ci, sap in enumerate(srcs):
                psA = pps.tile([P, F], F32, tag="psA")
                nc.tensor.matmul(psA[:], OH1[:], sap)
                prm = plp.tile([P, F], F32, tag="prm")
                s.activation(prm[:], psA[:], Act.Copy)
                v.scalar_tensor_tensor(junk2[:], ohf[:], 1.0, prm[:],
                                       op0=Alu.mult, op1=Alu.mult,
                                       accum_out=cX[:, ci:ci + 1])
            cV = cX[:, 0:1]
            cdx0 = cX[:, 1:2]
            cdx1 = cX[:, 2:3]
            cdy0 = cX[:, 3:4]
            cdy1 = cX[:, 4:5]
            cAn = cX[:, 5:6]
            clse = cX[:, 6:7]

            # ---------- candidate iou vs all 20 gts -> first argmax ----------
            q0 = pio.tile([P, M], F32, tag="q0")
            q1 = pio.tile([P, M], F32, tag="q1")
            iwm = pio.tile([P, M], F32, tag="iwm")
            iom = pio.tile([P, M], F32, tag="iom2")
            v.tensor_scalar(q0[:], bx0, cdx0, None, op0=Alu.max)
            v.tensor_scalar(q1[:], bx1, cdx1, None, op0=Alu.min)
            v.tensor_tensor(q1[:], q1[:], q0[:], op=Alu.subtract)
            v.tensor_scalar(iwm[:], q1[:], 0.0, None, op0=Alu.max)
            v.tensor_scalar(q0[:], by0, cdy0, None, op0=Alu.max)
            v.tensor_scalar(q1[:], by1, cdy1, None, op0=Alu.min)
            v.tensor_tensor(q1[:], q1[:], q0[:], op=Alu.subtract)
            v.tensor_scalar(q1[:], q1[:], 0.0, None, op0=Alu.max)
            v.tensor_tensor(iom[:], iwm[:], q1[:], op=Alu.mult)     # inter
            v.tensor_scalar(q0[:], bA[:], cAn, None, op0=Alu.add)   # S = An+Am
            v.reciprocal(q0[:], q0[:])
            v.tensor_tensor(iom[:], iom[:], q0[:], op=Alu.mult)     # r
            eqm = pio.tile([P, M], F32, tag="eqm")
            # tolerance match: |iou_c - mx| <= 1e-6 (bit-exact equality is
            # fragile across ACT-relu vs DVE-max rounding on HW)
            v.tensor_scalar(eqm[:], iom[:], cV, None, op0=Alu.subtract)
            v.tensor_tensor(eqm[:], eqm[:], eqm[:], op=Alu.mult)
            v.tensor_scalar(eqm[:], eqm[:], 1.0e-12, None, op0=Alu.is_le)
            v.scalar_tensor_tensor(eqm[:], eqm[:], -999.0, iota_m[:],
                                   op0=Alu.mult, op1=Alu.add)
            mstar = pio.tile([P, 1], F32, tag="mstar")
            v.tensor_reduce(mstar[:], eqm[:], axis=mybir.AxisListType.X, op=Alu.min)
            v.tensor_scalar(mstar[:], mstar[:], 999.0, None, op0=Alu.add)
            v.tensor_scalar(mstar[:], mstar[:], float(M - 1), None, op0=Alu.min)
            ohm = pio.tile([P, M], F32, tag="ohm")
            v.tensor_scalar(ohm[:], iota_m[:], mstar[:], None, op0=Alu.is_equal)
            cgt = pio.tile([P, 8], F32, tag="cgt")
            for gi, gap in enumerate((bx0, by0, bx1, by1, bA[:], tlf[:])):
                gjunk = plp.tile([P, M], F32, tag="gjunk")
                v.scalar_tensor_tensor(gjunk[:], ohm[:], 1.0, gap,
                                       op0=Alu.mult, op1=Alu.mult,
                                       accum_out=cgt[:, gi:gi + 1])
            cbx0 = cgt[:, 0:1]
            cby0 = cgt[:, 1:2]
            cbx1 = cgt[:, 2:3]
            cby1 = cgt[:, 3:4]
            cbA = cgt[:, 4:5]
            ctl = cgt[:, 5:6]

            # ---------- candidate cls value via class-masked accumulated permute ----------
            dtg = pdr.tile([1, P], F32, tag="dtg")
            nc.sync.dma_start(
                dtg[:].rearrange("o p -> (o p)").rearrange("(p o) -> p o", o=1), ctl)
            tgrow = pio.tile([P, P], F32, tag="tgrow")
            nc.sync.dma_start(tgrow[:], dtg[:].broadcast_to([P, P]))
            psC = pps.tile([P, F], F32, tag="psC")
            for c in range(C):
                ohct = plp.tile([P, P], F32, tag="ohct")
                v.tensor_scalar(ohct[:], tgrow[:], float(c), None, op0=Alu.is_equal)
                v.tensor_tensor(ohct[:], ohct[:], OH1[:], op=Alu.mult)
                nc.tensor.matmul(psC[:], ohct[:], cls[:, c, :],
                                 start=(c == 0), stop=(c == C - 1))
            clsPick = pio.tile([P, F], F32, tag="mxc2")
            s.activation(clsPick[:], psC[:], Act.Copy)
            ccls = pio.tile([P, 1], F32, tag="ccls")
            v.scalar_tensor_tensor(junk2[:], ohf[:], 1.0, clsPick[:],
                                   op0=Alu.mult, op1=Alu.mult, accum_out=ccls[:])

            # ---------- candidate pos focal ----------
            posf = pio.tile([P, 1], F32, tag="posf")
            v.tensor_scalar(posf[:], cV, taup[:], None, op0=Alu.is_ge)
            ce_p = pio.tile([P, 1], F32, tag="cep")
            v.tensor_tensor(ce_p[:], clse, ccls[:], op=Alu.subtract)
            pt_p = pio.tile([P, 1], F32, tag="ptp")
            s.activation(pt_p[:], ce_p[:], Act.Exp, scale=-1.0)
            u_p = pio.tile([P, 1], F32, tag="up")
            v.tensor_scalar(u_p[:], pt_p[:], -1.0, 1.0, op0=Alu.mult, op1=Alu.add)
            v.tensor_tensor(u_p[:], u_p[:], u_p[:], op=Alu.mult)
            foc_p = pio.tile([P, 1], F32, tag="focp")
            v.scalar_tensor_tensor(foc_p[:], u_p[:], ALPHA, ce_p[:],
                                   op0=Alu.mult, op1=Alu.mult)
            v.tensor_tensor(sums[:, 2:3], posf[:], foc_p[:], op=Alu.mult)

            # ---------- candidate giou ----------
            onemv = pio.tile([P, 1], F32, tag="onemv")      # 1 - V
            v.tensor_scalar(onemv[:], cV, -1.0, 1.0, op0=Alu.mult, op1=Alu.add)
            cun = pio.tile([P, 1], F32, tag="cun")
            ctt = pio.tile([P, 1], F32, tag="ctt")
            v.tensor_tensor(ctt[:], cAn, cbA, op=Alu.add)
            v.tensor_tensor(cun[:], onemv[:], ctt[:], op=Alu.mult)  # union = S*(1-V)
            iouv = pio.tile([P, 1], F32, tag="iouv")        # true iou = V/(1-V)
            v.reciprocal(iouv[:], onemv[:])
            v.tensor_tensor(iouv[:], iouv[:], cV, op=Alu.mult)
            ce0 = pio.tile([P, 1], F32, tag="ce0")
            ce1 = pio.tile([P, 1], F32, tag="ce1")
            cf0 = pio.tile([P, 1], F32, tag="cf0")
            cf1 = pio.tile([P, 1], F32, tag="cf1")
            v.tensor_tensor(ce0[:], cdx0, cbx0, op=Alu.min)
            v.tensor_tensor(ce1[:], cdx1, cbx1, op=Alu.max)
            v.tensor_tensor(ce1[:], ce1[:], ce0[:], op=Alu.subtract)
            v.tensor_tensor(cf0[:], cdy0, cby0, op=Alu.min)
            v.tensor_tensor(cf1[:], cdy1, cby1, op=Alu.max)
            v.tensor_tensor(cf1[:], cf1[:], cf0[:], op=Alu.subtract)
            cenc = pio.tile([P, 1], F32, tag="cenc")
            v.tensor_tensor(cenc[:], ce1[:], cf1[:], op=Alu.mult)
            cre = pio.tile([P, 1], F32, tag="cre")
            v.reciprocal(cre[:], cenc[:])
            v.tensor_tensor(cenc[:], cenc[:], cun[:], op=Alu.subtract)
            v.tensor_tensor(cenc[:], cenc[:], cre[:], op=Alu.mult)
            cgi = pio.tile([P, 1], F32, tag="cgi")
            v.tensor_tensor(cgi[:], iouv[:], cenc[:], op=Alu.subtract)
            v.tensor_scalar(cgi[:], cgi[:], -1.0, 1.0, op0=Alu.mult, op1=Alu.add)
            v.tensor_tensor(sums[:, 3:4], posf[:], cgi[:], op=Alu.mult)

            sumr = pio.tile([P, 4], F32, tag="sumr")
            psum_bcast(sumr[:], sums[:], 4)
            sel_sum = sumr[:, 0:1]
            allneg_sum = sumr[:, 1:2]
            pos_sum = sumr[:, 2:3]
            reg_sum = sumr[:, 3:4]

            # ---------- combine ----------
            branch = pio.tile([P, 1], F32, tag="branch")   # nneg > k
            v.tensor_scalar(branch[:], nneg, kk[:], None, op0=Alu.is_gt)
            negsum = pio.tile([P, 1], F32, tag="negsum")
            v.tensor_tensor(t1[:], sel_sum, allneg_sum, op=Alu.subtract)
            v.tensor_tensor(t1[:], t1[:], branch[:], op=Alu.mult)
            v.tensor_tensor(negsum[:], allneg_sum, t1[:], op=Alu.add)
            negcnt = pio.tile([P, 1], F32, tag="negcnt")
            v.tensor_tensor(t1[:], kk[:], nneg, op=Alu.subtract)
            v.tensor_tensor(t1[:], t1[:], branch[:], op=Alu.mult)
            v.tensor_tensor(negcnt[:], nneg, t1[:], op=Alu.add)
            tots = pio.tile([P, 1], F32, tag="tots")
            v.tensor_tensor(tots[:], num_pos[:], negcnt[:], op=Alu.add)
            v.tensor_scalar(tots[:], tots[:], 1.0, None, op0=Alu.max)
            v.reciprocal(tots[:], tots[:])
            clsl = pio.tile([P, 1], F32, tag="clsl")
            v.tensor_tensor(clsl[:], pos_sum, negsum[:], op=Alu.add)
            v.tensor_tensor(clsl[:], clsl[:], tots[:], op=Alu.mult)
            npc = pio.tile([P, 1], F32, tag="npc")
            v.tensor_scalar(npc[:], num_pos[:], 1.0, None, op0=Alu.max)
            v.reciprocal(npc[:], npc[:])
            regl = pio.tile([P, 1], F32, tag="regl")
            v.tensor_tensor(regl[:], reg_sum, npc[:], op=Alu.mult)
            v.tensor_tensor(clsl[:], clsl[:], regl[:], op=Alu.add)
            v.tensor_tensor(acc_part[:], acc_part[:], clsl[:], op=Alu.add)

        nc.sync.dma_start(o_part[:], acc_part[:1, 0:1])


# ======================= host-side runner =======================
_CACHE = {}


def _split_multiwaits(bj):
    """This container's walrus supports one sync-wait per instruction; split
    Tile's multi-wait instructions into NoOp chains at BIR-JSON level."""
    import json
    m = json.loads(bj)
    for fn in m["functions"]:
        for b in fn["blocks"]:
            out = []
            for i in b.get("instructions", []):
                si = i.get("sync_info") or {}
                ow = si.get("on_wait") or []
                if len(ow) > 1:
                    for w_ix, w in enumerate(ow[:-1]):
                        out.append({"name": f"{i['name']}_w{w_ix}",
                                    "opcode": "NoOp", "engine": i["engine"],
                                    "ins": [], "outs": [],
                                    "sync_info": {"on_wait": [w],
                                                  "on_update": []}})
                    si["on_wait"] = [ow[-1]]
                out.append(i)
            b["instructions"] = out
    return json.dumps(m).encode()


def _install_bir_patch():
    import concourse.bass2jax as b2j
    if getattr(b2j, "_mw_patched", False):
        return
    orig = b2j.compile_bir_kernel

    def patched(bir_json, tmpdir, neff_name="file.neff"):
        return orig(_split_multiwaits(bir_json), tmpdir, neff_name=neff_name)

    b2j.compile_bir_kernel = patched
    b2j._mw_patched = True


def _get_nc():
    if "nc" in _CACHE:
        return _CACHE["nc"]
    import concourse.tile as tile
    nc = bass.Bass("TRN2", target_bir_lowering=False, debug=False)
    d_cls = nc.dram_tensor("d_cls", [IMGS, C, N], F32, kind="ExternalInput").ap()
    d_reg = nc.dram_tensor("d_reg", [IMGS, 4, N], F32, kind="ExternalInput").ap()
    d_anch = nc.dram_tensor("d_anch", [N * 4], F32, kind="ExternalInput").ap()
    d_tb = nc.dram_tensor("d_tb", [IMGS, M, 4], F32, kind="ExternalInput").ap()
    d_tl = nc.dram_tensor("d_tl", [IMGS, M], I32, kind="ExternalInput").ap()
    d_sc = nc.dram_tensor("d_sc", [IMGS, N], F32, kind="ExternalInput").ap()
    d_out = nc.dram_tensor("d_out", [1, 1], F32, kind="ExternalOutput").ap()
    with tile.TileContext(nc) as tc:
        build(nc, tc, [d_out], [d_cls, d_reg, d_anch, d_tb, d_tl, d_sc])
    _CACHE["nc"] = nc
    return nc


def _in_maps(cls_output, reg_output, anchors, target_boxes, target_labels,
             neg_scores, n_cores=8):
    B = cls_output.shape[0]
    assert B == n_cores * IMGS
    maps = []
    for cix in range(n_cores):
        i0 = cix * IMGS
        sl = slice(i0, i0 + IMGS)
        maps.append({
            "d_cls": np.ascontiguousarray(
                np.asarray(cls_output[sl], np.float32).reshape(IMGS, C, N)),
            "d_reg": np.ascontiguousarray(
                np.asarray(reg_output[sl], np.float32).reshape(IMGS, 4, N)),
            "d_anch": np.ascontiguousarray(
                np.asarray(anchors, np.float32).reshape(N * 4)),
            "d_tb": np.ascontiguousarray(
                np.asarray(target_boxes[sl], np.float32)),
            "d_tl": np.ascontiguousarray(
                np.asarray(target_labels[sl]).astype(np.int32)),
            "d_sc": np.ascontiguousarray(
                np.asarray(neg_scores[sl], np.float32)),
        })
    return maps


def kernel(cls_output, reg_output, anchors, target_boxes, target_labels,
           neg_scores):
    from concourse.bass_utils import run_bass_kernel_spmd
    _install_bir_patch()
    nc = _get_nc()
    maps = _in_maps(cls_output, reg_output, anchors, target_boxes,
                    target_labels, neg_scores)
    res = run_bass_kernel_spmd(nc, maps, core_ids=list(range(8)))
    B = cls_output.shape[0]
    total = sum(float(r["d_out"][0, 0]) for r in res.results) / B
    return np.array(total, dtype=np.float32)

